# revision 1
# baseline (speedup 1.0000x reference)
import sys

sys.path.insert(0, "/root/problem")
import numpy as np

import qlin

_cache = {}


def _get_compiled(loop_n=1):
    key = ("nc", loop_n)
    if key not in _cache:
        _cache[key] = qlin.build_nc(loop_n)
    return _cache[key]


def _prep_in_maps(inputs):
    key = "ins"
    if key not in _cache:
        _cache[key] = qlin.prep_core_inputs(**inputs)
    return _cache[key]


def kernel(x, qweight, qzeros, scales, bias):
    inputs = dict(x=x, qweight=qweight, qzeros=qzeros, scales=scales, bias=bias)
    in_maps = qlin.prep_core_inputs(**inputs)
    nc = _get_compiled()
    if "runner" not in _cache:
        _cache["runner"] = qlin.Runner(nc)
    res = _cache["runner"].run(in_maps)
    return qlin.gather_output(res)
